# revision 33
# baseline (speedup 1.0000x reference)
"""Trainium2 Bass kernel for DeepJ biaxial LSTM (nn_DeepJ_335007449482).

Sharding: pure data parallelism — batch 1024 split as 128 rows per core
across 8 NeuronCores. Weights replicated. Full inputs in, full output out.

Math (identical to the reference):
  Stage A (time axis, per (b, n) independent; h=c=0 so Whh terms vanish
  and the f-gate is dead):   h = sig(o) * tanh(sig(i) * tanh(g))   x2 layers
  Stage A2: Gx0 = n_Wih0 @ [feats; cond] + n_b0 (input part of note-LSTM L0)
  Stage B: sequential 2-layer LSTM scan over n = 0..47, hidden 128,
  batch 128/core.  Out: sigmoid(out_W @ h2_n + out_b).

Layout: feature-major ([feature, batch]) throughout; weights are the
stationary matmul operand; biases ride the matmuls via an appended
ones-row on the moving operand.  All matmul operands bf16 (fp32 matmul is
4x slower on TRN2), PSUM accumulation fp32, LSTM cell state fp32.
"""

import numpy as np
import ml_dtypes

B, N, OCT, NOCT, TU, NU = 1024, 48, 12, 4, 256, 128
IN_T = 50
N_CORES = 8
BC = B // N_CORES          # 128 batch rows per core
ROWS = N * BC              # 6144 stage-A rows per core, (n, b) n-major
NCH = ROWS // 512          # 12 row-chunks of 512
BF16 = ml_dtypes.bfloat16
HB = ROWS                  # hidden-chunk block size in the big tiles


def _host_prep(inputs):
    f32 = np.float32
    ni = np.asarray(inputs["note_input"], f32)
    tg = np.asarray(inputs["targets"], f32)

    pitch_pos = np.arange(N, dtype=f32) / N
    pitch_class = np.tile(np.eye(OCT, dtype=f32), (NOCT, 1))
    chord = ni.reshape(B, OCT, NOCT).sum(-1)
    xp = np.pad(ni, ((0, 0), (OCT, OCT)))
    vic_idx = np.arange(N)[:, None] + np.arange(2 * OCT + 1)[None, :]
    vicinity = xp[:, vic_idx]
    rnn_in = np.concatenate(
        [
            np.broadcast_to(pitch_pos[None, :, None], (B, N, 1)),
            np.broadcast_to(pitch_class[None], (B, N, OCT)),
            vicinity,
            np.broadcast_to(chord[:, None, :], (B, N, OCT)),
        ],
        axis=-1,
    )  # [B, N, 50]
    cond = np.concatenate([np.zeros((B, 1), f32), tg[:, :-1]], axis=1)  # [B, N]

    # time-axis weights: keep gates [i, o, g] (f is dead), transposed, bias row
    sel = np.concatenate(
        [np.arange(TU), np.arange(3 * TU, 4 * TU), np.arange(2 * TU, 3 * TU)]
    )
    t_w0 = np.concatenate(
        [np.asarray(inputs["t_Wih0"], f32)[sel].T,
         np.asarray(inputs["t_b0"], f32)[sel][None]], 0)   # [51, 768]
    t_w1 = np.concatenate(
        [np.asarray(inputs["t_Wih1"], f32)[sel].T,
         np.asarray(inputs["t_b1"], f32)[sel][None]], 0)   # [257, 768]

    # note-axis weights: gate order [i, f, o, g]
    perm = np.concatenate(
        [np.arange(NU), np.arange(NU, 2 * NU),
         np.arange(3 * NU, 4 * NU), np.arange(2 * NU, 3 * NU)])
    nw0 = np.asarray(inputs["n_Wih0"], f32)[perm]          # [512, 257]
    n_w0ab = np.ascontiguousarray(nw0[:, :256].T)          # [256, 512]
    n_w0c = np.stack([nw0[:, 256], np.asarray(inputs["n_b0"], f32)[perm]])
    n_whh0 = np.asarray(inputs["n_Whh0"], f32)[perm].T     # [128, 512]
    n_wih1 = np.asarray(inputs["n_Wih1"], f32)[perm].T     # [128, 512]
    n_whh1 = np.asarray(inputs["n_Whh1"], f32)[perm].T     # [128, 512]
    n_b1 = np.asarray(inputs["n_b1"], f32)[perm][None]     # [1, 512]
    outw = np.asarray(inputs["out_W"], f32).T              # [128, 1]
    outb = float(np.asarray(inputs["out_b"], f32)[0])

    # tanh(x) = 2*sigmoid(2x) - 1: pre-scale all g-gate weight blocks by 2
    t_w0[:, 512:768] *= 2.0
    t_w1[:, 512:768] *= 2.0
    for w in (n_w0ab, n_whh0, n_wih1, n_whh1, n_b1):
        w[..., 384:512] *= 2.0
    n_w0c[:, 384:512] *= 2.0
    shared = {
        "t_w0": t_w0, "t_w1a": t_w1[0:128], "t_w1b": t_w1[128:256],
        "t_w1c": t_w1[256:257],
        "n_w0a": n_w0ab[0:128], "n_w0b": n_w0ab[128:256], "n_w0c": n_w0c,
        "n_whh0": n_whh0, "n_wih1": n_wih1, "n_whh1": n_whh1, "n_b1": n_b1,
        "outw": outw,
    }
    shared = {k: np.ascontiguousarray(v).astype(BF16) for k, v in shared.items()}

    in_maps = []
    ones_row = np.ones((1, ROWS), f32)
    for i in range(N_CORES):
        bs = slice(i * BC, (i + 1) * BC)
        rT = rnn_in[bs].transpose(2, 1, 0).reshape(IN_T, ROWS)      # [50, 6144]
        rT = np.concatenate([rT, ones_row], 0)                      # [51, 6144]
        condT = cond[bs].T.reshape(1, ROWS)
        co = np.concatenate([condT, ones_row], 0)                   # [2, 6144]
        m = dict(shared)
        m["rnnT"] = np.ascontiguousarray(rT).astype(BF16)
        m["condones"] = np.ascontiguousarray(co).astype(BF16)
        m["ones"] = ones_row.astype(BF16)
        m["outbrow"] = np.full((1, BC), outb, f32).astype(BF16)
        in_maps.append(m)
    return in_maps, outb


def _build(outb):
    import concourse.bacc as bacc
    import concourse.tile as tile
    from concourse import mybir

    F32, B16 = mybir.dt.float32, mybir.dt.bfloat16
    AF = mybir.ActivationFunctionType
    nc = bacc.Bacc("TRN2", target_bir_lowering=False, debug=False, num_devices=1)

    dp = nc.declare_dram_parameter
    d_rnnT = dp("rnnT", [51, ROWS], B16, isOutput=False)
    d_co = dp("condones", [2, ROWS], B16, isOutput=False)
    d_ones = dp("ones", [1, ROWS], B16, isOutput=False)
    d_tw0 = dp("t_w0", [51, 768], B16, isOutput=False)
    d_tw1a = dp("t_w1a", [128, 768], B16, isOutput=False)
    d_tw1b = dp("t_w1b", [128, 768], B16, isOutput=False)
    d_tw1c = dp("t_w1c", [1, 768], B16, isOutput=False)
    d_nw0a = dp("n_w0a", [128, 512], B16, isOutput=False)
    d_nw0b = dp("n_w0b", [128, 512], B16, isOutput=False)
    d_nw0c = dp("n_w0c", [2, 512], B16, isOutput=False)
    d_nwhh0 = dp("n_whh0", [128, 512], B16, isOutput=False)
    d_nwih1 = dp("n_wih1", [128, 512], B16, isOutput=False)
    d_nwhh1 = dp("n_whh1", [128, 512], B16, isOutput=False)
    d_nb1 = dp("n_b1", [1, 512], B16, isOutput=False)
    d_outw = dp("outw", [128, 1], B16, isOutput=False)
    d_outbrow = dp("outbrow", [1, BC], B16, isOutput=False)
    d_y = dp("y", [BC, N], F32, isOutput=True)

    with tile.TileContext(nc) as tc:
        with (
            tc.tile_pool(name="wts", bufs=1) as wts,
            tc.tile_pool(name="big", bufs=1) as big,
            tc.tile_pool(name="apsum", bufs=1, space="PSUM") as apsum,
            tc.tile_pool(name="spsum", bufs=1, space="PSUM") as spsum,
            tc.tile_pool(name="sa", bufs=2) as sa_pool,
            tc.tile_pool(name="ta", bufs=2) as ta_pool,
            tc.tile_pool(name="ca", bufs=2) as ca_pool,
            tc.tile_pool(name="tca", bufs=2) as tca_pool,
            tc.tile_pool(name="ss", bufs=2) as ss_pool,
            tc.tile_pool(name="st", bufs=2) as st_pool,
            tc.tile_pool(name="stf", bufs=2) as stf_pool,
            tc.tile_pool(name="stc", bufs=2) as stc_pool,
            tc.tile_pool(name="yo", bufs=1) as yo_pool,
        ):
            # ---------------- constants / inputs ----------------
            def load(dram, shape, tag, eng=None):
                t = wts.tile(shape, B16, tag=tag)
                (eng or nc.sync).dma_start(t[:], dram[:])
                return t

            tw0 = load(d_tw0, [51, 768], "tw0")
            rnnT = wts.tile([51, ROWS], B16, tag="rnnT")
            nc.sync.dma_start(rnnT[:, 0:1024], d_rnnT[:, 0:1024])
            co = load(d_co, [2, ROWS], "condones")
            ones = load(d_ones, [1, ROWS], "ones")
            tw1a = load(d_tw1a, [128, 768], "tw1a")
            tw1b = load(d_tw1b, [128, 768], "tw1b")
            tw1c = load(d_tw1c, [1, 768], "tw1c")
            nw0a = load(d_nw0a, [128, 512], "nw0a")
            nw0b = load(d_nw0b, [128, 512], "nw0b")
            nw0c = load(d_nw0c, [2, 512], "nw0c")
            nwhh0 = load(d_nwhh0, [128, 512], "nwhh0")
            nwih1 = load(d_nwih1, [128, 512], "nwih1")
            nwhh1 = load(d_nwhh1, [128, 512], "nwhh1")
            nb1 = load(d_nb1, [1, 512], "nb1")
            outw = load(d_outw, [128, 1], "outw")
            outbrow = load(d_outbrow, [1, BC], "outbrow")
            nc.sync.dma_start(rnnT[:, 1024:ROWS], d_rnnT[:, 1024:ROWS])

            # ---------------- persistent activations ----------------
            h1T = big.tile([128, 2 * HB], B16, tag="h1T")   # (hc, rows)
            featsT = big.tile([128, 2 * HB], B16, tag="featsT")
            HH = big.tile([128, 2 * HB], B16, tag="HH")     # h1 | h2 per note
            C = big.tile([128, 256], B16, tag="C")          # c1 | c2

            MM = nc.tensor.matmul

            # ===== Stage A (256-row half-chunks) software-pipelined against
            # the scan at sub-step granularity.  All engines are in-order, so
            # emission order == per-engine execution order; stage-A ACT work
            # is split so each piece fits inside a scan dependency gap. =====

            def a_l0_mms(hc):
                cs = slice(hc * 256, (hc + 1) * 256)
                ps = apsum.tile([128, 1536], F32, tag="ap")
                for mc in range(6):
                    MM(ps[:, mc * 256:(mc + 1) * 256],
                       tw0[:, mc * 128:(mc + 1) * 128], rnnT[:, cs],
                       start=True, stop=True)
                return ps

            def a_l1_mms(hc):
                cs = slice(hc * 256, (hc + 1) * 256)
                ps = apsum.tile([128, 1536], F32, tag="ap")
                h1v = h1T[:].rearrange("p (t x) -> p t x", x=HB)
                for mc in range(6):
                    ms = slice(mc * 128, (mc + 1) * 128)
                    os_ = slice(mc * 256, (mc + 1) * 256)
                    MM(ps[:, os_], tw1a[:, ms], h1v[:, 0, cs], start=True, stop=False)
                    MM(ps[:, os_], tw1b[:, ms], h1v[:, 1, cs], start=False, stop=False)
                    MM(ps[:, os_], tw1c[:, ms], ones[:, cs], start=False, stop=True)
                return ps

            def a_sig1(ps):
                sg = sa_pool.tile([128, 1536], B16, tag="sa")
                nc.scalar.activation(sg[:, 0:768], ps[:, 0:768], AF.Sigmoid)
                return sg

            def a_sig2(ps, sg):
                nc.scalar.activation(sg[:, 768:1536], ps[:, 768:1536], AF.Sigmoid)

            def a_rest(ps, sg, dst_big, hc):
                cs = slice(hc * 256, (hc + 1) * 256)
                tg_ = ta_pool.tile([128, 512], B16, tag="ta")
                nc.vector.tensor_scalar(tg_[:], sg[:, 1024:1536], 2.0, -1.0,
                                        mybir.AluOpType.mult, mybir.AluOpType.add)
                cc = ca_pool.tile([128, 512], B16, tag="ca")
                nc.vector.tensor_mul(cc[:], sg[:, 0:512], tg_[:])
                tc_ = tca_pool.tile([128, 512], B16, tag="tca")
                nc.scalar.activation(tc_[:], cc[:], AF.Tanh)
                dst = dst_big[:].rearrange("p (t x) -> p t x", x=HB)[:, :, cs]
                nc.vector.tensor_mul(
                    dst,
                    sg[:, 512:1024].rearrange("p (t x) -> p t x", x=256),
                    tc_[:].rearrange("p (t x) -> p t x", x=256))

            def ss_mms(k):
                do0, do1 = k < N, k >= 1
                ns = slice(k * BC, (k + 1) * BC)
                ps = spsum.tile([128, 1024], F32, tag="sp")
                # h1-independent matmuls first (in-order engines!)
                if do0:
                    for g in range(4):
                        gs = slice(g * 128, (g + 1) * 128)
                        MM(ps[:, gs], nw0a[:, gs], featsT[:, ns],
                           start=(g == 0), stop=False)
                        MM(ps[:, gs], nw0b[:, gs],
                           featsT[:, HB + k * BC:HB + (k + 1) * BC],
                           start=False, stop=False)
                        MM(ps[:, gs], nw0c[:, gs], co[:, ns],
                           start=False, stop=(k == 0 and g == 3))
                if do1:
                    for g in range(4):
                        gs = slice(g * 128, (g + 1) * 128)
                        os_ = slice(512 + g * 128, 512 + (g + 1) * 128)
                        MM(ps[:, os_], nb1[:, gs], ones[:, 0:BC],
                           start=(g == 0), stop=False)
                        if k >= 2:
                            h2p = HH[:, HB + (k - 2) * BC:HB + (k - 1) * BC]
                            MM(ps[:, os_], nwhh1[:, gs], h2p, start=False, stop=False)
                # h1-dependent tail
                if do0 and k >= 1:
                    h1p = HH[:, (k - 1) * BC:k * BC]
                    for g in range(4):
                        MM(ps[:, g * 128:(g + 1) * 128],
                           nwhh0[:, g * 128:(g + 1) * 128], h1p,
                           start=False, stop=(g == 3))
                if do1:
                    h1p = HH[:, (k - 1) * BC:k * BC]
                    for g in range(4):
                        gs = slice(g * 128, (g + 1) * 128)
                        os_ = slice(512 + g * 128, 512 + (g + 1) * 128)
                        MM(ps[:, os_], nwih1[:, gs], h1p, start=False, stop=(g == 3))
                return ps

            def ss_sigma0(k, ps):
                S = ss_pool.tile([128, 1024], B16, tag="ss")  # [ifoG0 | ifoG1]
                if 2 <= k <= 47:
                    nc.scalar.activation(S[:], ps[:], AF.Sigmoid)
                elif k < N:
                    nc.scalar.activation(S[:, 0:512], ps[:, 0:512], AF.Sigmoid)
                return S

            def ss_sigma1(k, ps, S):
                if k >= 1 and not (2 <= k <= 47):
                    nc.scalar.activation(S[:, 512:1024], ps[:, 512:1024], AF.Sigmoid)

            def ss_tail(k, S, layers):
                """Per-layer LSTM cell updates; layer 0 first (its h1 is the
                recurrence-critical output), layer 1 has a superstep of slack."""
                AOP = mybir.AluOpType
                T = st_pool.tile([128, 256], B16, tag="st")
                Tc = stc_pool.tile([128, 256], B16, tag="stc")
                tf = stf_pool.tile([128, 256], B16, tag="stf")

                def cell(layer):
                    sb, cb = 512 * layer, 128 * layer
                    note = k - layer
                    fresh = (k == layer)  # first step of this layer: c_prev=0
                    si = S[:, sb:sb + 128]
                    sf = S[:, sb + 128:sb + 256]
                    so = S[:, sb + 256:sb + 384]
                    G = S[:, sb + 384:sb + 512]
                    Ts = T[:, cb:cb + 128]
                    Cs = C[:, cb:cb + 128]
                    Tcs = Tc[:, cb:cb + 128]
                    nc.vector.tensor_scalar(Ts, G, 2.0, -1.0, AOP.mult, AOP.add)
                    if not fresh:
                        tfs = tf[:, cb:cb + 128]
                        nc.vector.tensor_mul(tfs, sf, Cs)
                    nc.vector.tensor_mul(Cs, si, Ts)
                    if not fresh:
                        nc.vector.tensor_add(Cs, Cs, tfs)
                    nc.scalar.activation(Tcs, Cs, AF.Tanh)
                    nc.vector.tensor_mul(
                        HH[:, layer * HB + note * BC:layer * HB + (note + 1) * BC],
                        so, Tcs)

                if 2 <= k <= 43:
                    if 0 in layers:
                        # merged both-layer ops: minimal ACT/DVE busy for the
                        # ACT-bound mixed regime
                        Sv = S[:].rearrange("p (t x) -> p t x", x=512)
                        Tv = T[:].rearrange("p (t x) -> p t x", x=128)
                        Cv = C[:].rearrange("p (t x) -> p t x", x=128)
                        tfv = tf[:].rearrange("p (t x) -> p t x", x=128)
                        nc.vector.tensor_scalar(Tv, Sv[:, :, 384:512], 2.0, -1.0,
                                                AOP.mult, AOP.add)
                        nc.vector.tensor_mul(tfv, Sv[:, :, 128:256], Cv)
                        nc.vector.tensor_mul(Cv, Sv[:, :, 0:128], Tv)
                        nc.vector.tensor_add(C[:], C[:], tf[:])
                        nc.scalar.activation(Tc[:], C[:], AF.Tanh)
                        nc.vector.tensor_mul(HH[:, k * BC:(k + 1) * BC],
                                             S[:, 256:384], Tc[:, 0:128])
                        nc.vector.tensor_mul(HH[:, HB + (k - 1) * BC:HB + k * BC],
                                             S[:, 768:896], Tc[:, 128:256])
                    return
                if 0 in layers and k < N:
                    cell(0)
                if 1 in layers and k >= 1:
                    cell(1)

            NHC = ROWS // 256
            proj = {}

            def proj_mms(lo, hi):
                if "ps" not in proj:
                    proj["ps"] = apsum.tile([128, 512], F32, tag="ap", name="projps")
                pp = proj["ps"]
                for n in range(lo, hi):
                    MM(pp[:, n:n + 1], HH[:, HB + n * BC:HB + (n + 1) * BC],
                       outw[:], start=(n == 0), stop=False)

            for j in range(NHC + 2):
                for phase in (0, 1):
                    # phase 0: L0(j) vs superstep 2j-4; phase 1: L1(j-1) vs 2j-3
                    if phase == 0:
                        a_on = j < NHC
                        hc, mmfn, dst = j, a_l0_mms, h1T
                    else:
                        a_on = 1 <= j <= NHC
                        hc, mmfn, dst = j - 1, a_l1_mms, featsT
                    k = 2 * j - 4 + phase
                    s_on = 0 <= k < N
                    if a_on:
                        psA = mmfn(hc)
                    if s_on:
                        psS = ss_mms(k)
                    if k == 46:
                        proj_mms(0, 16)
                    elif k == 47:
                        proj_mms(16, 32)
                    if s_on:
                        S = ss_sigma0(k, psS)
                        ss_tail(k, S, (0,))
                        ss_sigma1(k, psS, S)
                        ss_tail(k, S, (1,))
                    if a_on:
                        sgA = a_sig1(psA)
                        a_sig2(psA, sgA)
                        a_rest(psA, sgA, dst, hc)
            psS = ss_mms(N)
            proj_mms(32, 46)
            S = ss_sigma0(N, psS)
            ss_sigma1(N, psS, S)
            ss_tail(N, S, (0, 1))

            # ==================== output projection tail ====================
            pp = proj["ps"]
            for n in range(46, N):
                MM(pp[:, n:n + 1], HH[:, HB + n * BC:HB + (n + 1) * BC],
                   outw[:], start=False, stop=False)
            MM(pp[:, 0:N], outbrow[:], ones[:, 0:N], start=False, stop=True)
            Y = yo_pool.tile([128, N], F32, tag="yo")
            nc.scalar.activation(Y[:], pp[:, 0:N], AF.Sigmoid)
            nc.sync.dma_start(d_y[:], Y[:])

    nc.compile()
    return nc


_CACHE = {}


def _get_program(outb):
    key = round(outb, 10)
    if key not in _CACHE:
        _CACHE[key] = _build(outb)
    return _CACHE[key]


def kernel(**inputs) -> np.ndarray:
    from concourse.bass_utils import run_bass_kernel_spmd

    in_maps, outb = _host_prep(inputs)
    nc = _get_program(outb)
    res = run_bass_kernel_spmd(nc, in_maps, list(range(N_CORES)))
    return np.concatenate([np.asarray(res.results[i]["y"], np.float32)
                           for i in range(N_CORES)], axis=0)
